# revision 23
# baseline (speedup 1.0000x reference)
"""DynamicFilter Trainium2 kernel (fp16 streaming version).

Computation (per sample b):
    h  = tanh(query @ W1.T + b1)                      [B, 256]
    cw = (h @ W2.T + b2).reshape(B, C=32, K=31)       per-sample conv weights
    x[b,t,c] = sum_k cw[b,c,k] * pad(prev_attn)[b, t+k]
    out[b,t,o] = sum_c Wfc[o,c] x[b,t,c] + bfc[o]

Key algebraic fusion: fold the fc into the conv,
    Weff[b,o,k] = sum_c Wfc[o,c] cw[b,c,k]            [B, 128, 31]
    out[b,t,o]  = sum_k Weff[b,o,k] pad(prev_attn)[b, t+k] + bfc[o]
so the T-sized work is ONE matmul per (sample, 512-wide t-chunk):
    psum[128 o, 512 t] = WeffT_b[31, 128 o].T @ windows[31, 512 t]
The psum drain applies +bfc and the fp16 downcast in one op, alternated
across the Vector/Act engines (GPSIMD cannot read PSUM). b1 is applied
as the tanh activation bias; b2 via a ones-row K=1 accumulating matmul.

The kernel is DMA-engine bound on bytes (16 SDMA engines x ~22.5 B/ns)
and PE-bound on the main loop (1 moving column/cycle at the observed
1.2 GHz, 64 x 512-col matmuls = ~27us), so every big stream is fp16:
the shifted-replica windows (host-prepacked), the hypernet weights, and
the output (host upcasts to f32). fp16 keeps the global rel-err ~6e-4,
far inside the 2e-2 budget, and halves bytes vs an f32 pipeline. DMA
rows are >=1KB (mostly 4KB) so no 512B read-modify-write penalty.

Scheduling notes (from perfetto/NTFF traces):
 - ~7.2us fixed framework preamble before user instructions; input DMA
   dispatches can only begin then.
 - The scalar ring carries ONLY qt+w1 so (a) w1 gets most of the early
   DMA bandwidth (it gates mm1) and (b) the scalar engine reaches the
   tanh activations without queueing behind dispatches. Sync carries
   small operands first, then w2, then replicas, then the out-DMAs.
 - mm2 emits cw in host-permuted (k,c) column order with a 32-wide
   zero-padded stationary, so a DVE 32x32 StreamTranspose straight off
   the mm2 psum (with the f32->fp16 cast) yields cwT[c, 32k+b]; the
   weff matmuls then read their stationary via a stride-32 column
   slice. No gather DMA on the critical path.
 - psum drains are [128, 1024] (2 banks) ops alternating DVE/ACT; out
   tiles are [128, 2048] (4KB DRAM rows), dispatched on sync.

Sharding: data-parallel over batch. 64 samples / 8 cores = 8 per core.
Weights replicated. Output written [b, o, t] fp16 (4KB contiguous DMA
runs); host returns a transposed f32 view [B, T, O].
"""

import sys

import numpy as np

if "/opt/trn_rl_repo" not in sys.path:
    sys.path.insert(0, "/opt/trn_rl_repo")

from contextlib import ExitStack

import concourse.bass as bass
import concourse.mybir as mybir
import concourse.tile as tile
from concourse import bacc
from concourse.ap import AP
from concourse.bass_utils import run_bass_kernel_spmd

# Problem shapes (hardcoded per contract).
B, T = 64, 4096
D, H = 1024, 256
C, K, O = 32, 31, 128
PAD = (K - 1) // 2  # 15
NCORES = 8
BPC = B // NCORES  # 8 samples per core
TCH = 512  # t-chunk (matmul moving free dim; one psum bank)
NT = T // TCH  # 8 chunks per sample
OCH = 2048  # out-DMA column chunk (4KB fp16 rows)
NOCH = T // OCH
GROUPS = [(0, 3), (3, 3), (6, 2)]  # (first sample, count) per replica tile

F32 = mybir.dt.float32
F16 = mybir.dt.float16
AF = mybir.ActivationFunctionType

_CACHED = {}


def _build_nc(use_f32r=True):
    nc = bacc.Bacc(
        "TRN2", target_bir_lowering=False, debug=False, num_devices=NCORES
    )

    qT_h = nc.dram_tensor("qtp", [128, 8 * BPC], F16, kind="ExternalInput")
    rep_h = nc.dram_tensor("paRep", [len(GROUPS), 96, T], F16,
                           kind="ExternalInput")
    w1t_h = nc.dram_tensor("w1tp", [128, 8 * H], F16, kind="ExternalInput")
    b1_h = nc.dram_tensor("b1p", [128, 2], F32, kind="ExternalInput")
    w2t_h = nc.dram_tensor("w2tp", [128, 2 * C * K], F16, kind="ExternalInput")
    b2_h = nc.dram_tensor("b2", [C * K], F16, kind="ExternalInput")
    wfct_h = nc.dram_tensor("wfct3", [96, O], F16, kind="ExternalInput")
    bfcc_h = nc.dram_tensor("bfcc", [O, 1], F32, kind="ExternalInput")
    out_h = nc.dram_tensor("out", [BPC, O, T], F16, kind="ExternalOutput")

    with tile.TileContext(nc) as tc:
        _emit(tc, qT_h, rep_h, w1t_h, b1_h, w2t_h, b2_h, wfct_h, bfcc_h, out_h)

    nc.compile()
    return nc


def _emit(tc, qT_h, rep_h, w1t_h, b1_h, w2t_h, b2_h, wfct_h, bfcc_h, out_h):
    nc = tc.nc
    with ExitStack() as ctx:
        singles = ctx.enter_context(tc.tile_pool(name="singles", bufs=1))
        weff_pool = ctx.enter_context(tc.tile_pool(name="weff", bufs=3))
        pa_pool = ctx.enter_context(tc.tile_pool(name="pa", bufs=3))
        out_pool = ctx.enter_context(tc.tile_pool(name="outsb", bufs=6))
        head_ctx = ExitStack()
        psum_head = head_ctx.enter_context(
            tc.tile_pool(name="psum_head", bufs=2, space="PSUM")
        )

        # ---- staging ------------------------------------------------
        rep_ap = rep_h.ap()
        pa_tiles = [
            pa_pool.tile([96, T], F16, tag="pa", name=f"pa_g{g}")
            for g in range(len(GROUPS))
        ]

        # scalar ring: only what mm1 needs; w1's second half rides at the
        # head of the sync ring so w1 streams at both rings' head-of-line
        qt_sb = singles.tile([128, 8 * BPC], F16)
        nc.scalar.dma_start(qt_sb[:], qT_h.ap())
        w1t_tiles = []
        for ch in range(2):
            w1c = singles.tile([128, 1024], F16, name=f"w1c{ch}")
            eng = nc.scalar if ch == 0 else nc.sync
            eng.dma_start(
                w1c[:], w1t_h.ap()[:, 1024 * ch : 1024 * ch + 1024]
            )
            w1t_tiles.append(w1c)

        # sync ring: tiny operands, then w2, then replicas, then outs
        b1p_sb = singles.tile([128, 2], F32)
        nc.sync.dma_start(b1p_sb[:], b1_h.ap())
        wfct_sb = singles.tile([96, O], F16)
        nc.sync.dma_start(wfct_sb[:], wfct_h.ap())
        b2_sb = singles.tile([1, C * K], F16)
        nc.sync.dma_start(b2_sb[:], b2_h.ap().unsqueeze(0))
        bfcc_sb = singles.tile([O, 1], F32)
        nc.sync.dma_start(bfcc_sb[:], bfcc_h.ap())
        w2t_tiles = []
        for ch in range(2):
            w2c = singles.tile([128, C * K], F16, name=f"w2c{ch}")
            nc.sync.dma_start(
                w2c[:], w2t_h.ap()[:, C * K * ch : C * K * ch + C * K]
            )
            w2t_tiles.append(w2c)
        for gi, (b0, cnt) in enumerate(GROUPS):
            for ch in range(2):
                nc.sync.dma_start(
                    pa_tiles[gi][:, OCH * ch : OCH * ch + OCH],
                    rep_ap[gi, :, OCH * ch : OCH * ch + OCH],
                )

        ones_f32 = singles.tile([1, C], F32)
        nc.gpsimd.memset(ones_f32[:], 1.0)
        ones_sb = singles.tile([1, C], F16)
        nc.vector.tensor_copy(ones_sb[:], ones_f32[:])

        # ---- hypernet mm1 (transposed): hT[j, b] = sum_d W1T[d,j] qT[d,b]
        # htr[:, 32*jc : 32*jc+8] = tanh(hT chunk + b1); columns 8..31 of
        # each slot stay zero so mm2's 32-wide stationary is padded clean
        htr_tiles = []
        for jc in range(2):
            hjc = singles.tile([128, C], F16, name=f"htr{jc}")
            nc.gpsimd.memset(hjc[:], 0.0)
            htr_tiles.append(hjc)
        for jc in range(2):
            phT = psum_head.tile([128, BPC], F32, tag="pre")
            for dc in range(8):
                nc.tensor.matmul(
                    phT[:],
                    lhsT=w1t_tiles[dc // 4][
                        :, H * (dc % 4) + 128 * jc : H * (dc % 4) + 128 * jc + 128
                    ],
                    rhs=qt_sb[:, BPC * dc : BPC * dc + BPC],
                    start=(dc == 0),
                    stop=(dc == 7),
                )
            nc.scalar.activation(
                htr_tiles[jc][:, 0:BPC], phT[:], AF.Tanh,
                bias=b1p_sb[:, jc : jc + 1],
            )

        # ---- hypernet mm2: cwB[b, 32k + c] = sum_h W2kc[h, kc] hT[h, b] ----
        # W2 host-packed in (k, c) order; DVE StreamTranspose straight off
        # the psum (f32 -> fp16) lands cwT[c, 32k + b].
        cwB_sb = singles.tile([C, C * K], F16)
        # 32 extra zero columns = a 32nd all-zero tap so the weff matmul
        # writes all 32 psum rows (keeps the drain read race-free)
        cwT_sb = singles.tile([C, C * K + C], F16)
        nc.gpsimd.memset(cwT_sb[:, C * K : C * K + C], 0.0)
        SPLITS = [(0, 512), (512, 480)]
        for si, (off, width) in enumerate(SPLITS):
            pc = psum_head.tile([C, 512], F32, tag="pre")
            nc.tensor.matmul(  # bias row first: no htr dependency
                pc[:, 0:width],
                lhsT=ones_sb[:],
                rhs=b2_sb[:, off : off + width],
                start=True,
                stop=False,
            )
            for hc in range(2):
                nc.tensor.matmul(
                    pc[:, 0:width],
                    lhsT=htr_tiles[hc][:],
                    rhs=w2t_tiles[hc][:, off : off + width],
                    start=False,
                    stop=(hc == 1),
                )
            if si == 1:  # split the drains across DVE/ACT
                nc.scalar.activation(cwB_sb[:, off : off + width],
                                     pc[:, 0:width], AF.Identity)
            else:
                nc.vector.tensor_copy(cwB_sb[:, off : off + width],
                                      pc[:, 0:width])
        nc.vector.transpose(cwT_sb[:, 0 : C * K], cwB_sb[:])

        # ---- Weff per group: WeffT_b[k, o] = sum_c cw_b[c, k] WfcT[c, o] ---
        weff_tiles = []
        for gi, (b0, cnt) in enumerate(GROUPS):
            pw = psum_head.tile([96, O], F32, tag="pre")
            for i in range(cnt):
                b = b0 + i
                nc.tensor.matmul(
                    pw[32 * i : 32 * i + C, :],
                    lhsT=cwT_sb[0:C, b : C * K + C : C],
                    rhs=wfct_sb[0:C, :],
                    start=True,
                    stop=True,
                )
            wg = weff_pool.tile([96, O], F16, tag="weff")
            nc.vector.tensor_copy(
                wg[0 : 32 * cnt, :], pw[0 : 32 * cnt, :]
            )
            weff_tiles.append(wg)

        head_ctx.close()  # release head psum banks for the main pool
        psum_main = ctx.enter_context(
            tc.tile_pool(name="psum_main", bufs=4, space="PSUM")
        )

        # ---- main loop ------------------------------------------------
        # per (sample, 512-chunk): one [31,128]x[31,512] matmul into half
        # of a 2-bank psum tile; one merged [128,1024] drain per tile
        # alternating DVE/ACT; out-DMA per 2048 columns on sync.
        idx = 0
        out_ap = out_h.ap()
        for gi, (b0, cnt) in enumerate(GROUPS):
            pa_g = pa_tiles[gi]
            wg = weff_tiles[gi]
            for i in range(cnt):
                lhsT = wg[32 * i : 32 * i + K, :]
                b = b0 + i
                for oc in range(NOCH):
                    osb = out_pool.tile([O, OCH], F16, tag="osb")
                    for half in range(OCH // (2 * TCH)):
                        pm = psum_main.tile([O, 2 * TCH], F32, tag="pmm")
                        for sc in range(2):
                            tcn = oc * (OCH // TCH) + 2 * half + sc
                            nc.tensor.matmul(
                                pm[:, TCH * sc : TCH * sc + TCH],
                                lhsT=lhsT,
                                rhs=pa_g[32 * i : 32 * i + K,
                                         TCH * tcn : TCH * tcn + TCH],
                                start=True,
                                stop=True,
                            )
                        dst = osb[:, 2 * TCH * half : 2 * TCH * half + 2 * TCH]
                        if idx % 2 == 1:
                            nc.scalar.activation(
                                dst, pm[:], AF.Identity, bias=bfcc_sb[:, 0:1],
                            )
                        else:
                            nc.vector.tensor_scalar_add(
                                dst, pm[:], bfcc_sb[:, 0:1],
                            )
                        idx += 1
                    if b == BPC - 1 and oc == NOCH - 1:
                        # split the final DMA so its transfer overlaps
                        # the last drain
                        for hh in range(2):
                            nc.sync.dma_start(
                                out_ap[b, :, OCH * oc + 1024 * hh :
                                       OCH * oc + 1024 * hh + 1024],
                                osb[:, 1024 * hh : 1024 * hh + 1024],
                            )
                    else:
                        nc.sync.dma_start(
                            out_ap[b, :, OCH * oc : OCH * oc + OCH], osb[:]
                        )


def get_nc(use_f32r=True):
    key = ("nc",)
    if key not in _CACHED:
        _CACHED[key] = _build_nc(use_f32r)
    return _CACHED[key]


def make_in_maps(query, prev_attn, W1, b1, W2, b2, Wfc, bfc):
    """Shard + lay out host inputs for the 8 cores (fp16 streams)."""
    f = np.float32
    h16 = np.float16
    w1t = np.asarray(W1, f).T.astype(h16)  # [D, H]
    w2t0 = np.asarray(W2, f).T.astype(h16)  # [H, C*K] in (c,k) order
    # repack to (k,c) order: w2t[:, 32k + c] = W2T[:, 31c + k]
    w2t = np.ascontiguousarray(
        w2t0.reshape(H, C, K).transpose(0, 2, 1).reshape(H, C * K)
    )
    wfct1 = np.asarray(Wfc, f).T.astype(h16)  # [C, O]
    wfct = np.ascontiguousarray(np.concatenate([wfct1] * 3, axis=0))  # [96, O]
    b1p = np.ascontiguousarray(np.asarray(b1, f).reshape(2, 128).T)
    b2 = np.ascontiguousarray(
        np.asarray(b2, f).astype(h16).reshape(C, K).T.reshape(C * K)
    )
    bfcc = np.ascontiguousarray(np.asarray(bfc, f)[:, None])
    query = np.asarray(query, f).astype(h16)
    prev_attn = np.asarray(prev_attn, f).astype(h16)

    # prepack into the SBUF partition-major images the kernel DMAs verbatim
    w1tp = np.ascontiguousarray(
        w1t.reshape(8, 128, H).transpose(1, 0, 2).reshape(128, 8 * H)
    )
    w2tp = np.ascontiguousarray(
        w2t.reshape(2, 128, C * K).transpose(1, 0, 2).reshape(128, 2 * C * K)
    )

    in_maps = []
    for i in range(NCORES):
        sl = slice(i * BPC, (i + 1) * BPC)
        qT = query[sl].T  # [D, BPC]
        qtp = np.ascontiguousarray(
            qT.reshape(8, 128, BPC).transpose(1, 0, 2).reshape(128, 8 * BPC)
        )
        # shifted replicas: paRep[g, 32*j + k, t] = pad(prev_attn)[b0+j, k+t]
        padded = np.zeros((BPC, T + 2 * PAD), h16)
        padded[:, PAD : PAD + T] = prev_attn[sl]
        win = np.lib.stride_tricks.sliding_window_view(padded, T, axis=1)
        rep = np.zeros((len(GROUPS), 96, T), h16)
        for g, (b0, cnt) in enumerate(GROUPS):
            for j in range(cnt):
                rep[g, 32 * j : 32 * j + K] = win[b0 + j, :K]
        in_maps.append(
            {
                "qtp": qtp,
                "paRep": rep,
                "w1tp": w1tp,
                "b1p": b1p,
                "w2tp": w2tp,
                "b2": b2,
                "wfct3": wfct,
                "bfcc": bfcc,
            }
        )
    return in_maps


def assemble_output(results):
    """[8 cores] x [BPC, O, T] fp16 -> [B, T, O] f32 view."""
    full = np.concatenate([r["out"] for r in results], axis=0)  # [B, O, T]
    return full.astype(np.float32).transpose(0, 2, 1)


def kernel(query, prev_attn, W1, b1, W2, b2, Wfc, bfc):
    nc = get_nc()
    in_maps = make_in_maps(query, prev_attn, W1, b1, W2, b2, Wfc, bfc)
    res = run_bass_kernel_spmd(nc, in_maps, list(range(NCORES)))
    return assemble_output(res.results)
